# revision 5
# baseline (speedup 1.0000x reference)
"""Baichuan sliding-window GQA attention block on 8 trn2 NeuronCores.

Sharding: data-parallel over batch (2) x tensor-parallel over heads (4).
Core c handles batch b=c//4, head group g=c%4 (q heads 4g..4g+3, kv heads
2g..2g+1). Each core computes the fused qkv projection, RoPE, 2-tap causal
conv, windowed attention and a row-sharded o_proj partial; the host sums
the 4 partials per batch.

Performance structure (295us -> 229us vs the fp32 v1 kernel):
  - all SBUF/DRAM storage is bf16 (PSUM accumulation stays fp32): halves
    DMA traffic and runs packed DVE ops at 2-4x rate. Matmuls keep the
    1 cycle/row PE rate they had with f32r.
  - host-blocked DRAM layouts (hidden chunk-k-blocked, W_pack col-pair-k
    blocked, output u-oc-blocked) so every transfer is one large DMA;
    the HWDGE fixed overhead (~625ns/DMA) makes small DMAs expensive.
  - softmax denominators come from stationary-pb matmuls (pb chunk as the
    128-wide stationary operand, ones as a 1-row moving operand), which
    cost ~1 PE cycle each instead of streaming 512 rows per ones-matmul.
    The [128q, col] sums are reciprocated, flattened to partition 0 by a
    tiny SBUF->SBUF DMA, partition-broadcast on GPSIMD, and applied as
    per-128-query scaling multiplies with stride-8 access patterns.
  - one software-pipelined loop: QKV(t) chains carry the scores(t-1)
    matmuls between their contraction steps (paced at the Activation
    engine's exp rate), followed by PV(t-1)+sums, the per-kv-head
    normalization chains, and o_proj(t-2). scores(7) start inside
    iteration 7 and the tail interleaves o_proj(6) around PV(7) so the
    exp drain and the normalization latency stay off the critical path.
  - masks are added in-place into the scores PSUM only on the affected
    half-tiles; window-edge tiles compute/exp only their valid halves.
  - one accumulation chain per PSUM bank at a time: interleaving chains
    in a bank silently corrupts earlier chains' partials on hardware
    (verified empirically), so qkv columns and sum columns each run as
    contiguous chains.
  - a short warm-up matmul chain on scratch data ramps the PE out of its
    low p-state while the first weight DMAs land.
"""

import numpy as np
import ml_dtypes

B, S, H = 2, 2048, 2048
NH, NKV, HD = 16, 8, 128
WINDOW = 1024
THETA = 100000.0
TP = 4                      # tensor-parallel ways (head groups)
QH = NH // TP               # 4 q heads per core
KVH = NKV // TP             # 2 kv heads per core
NCORES = 8
SCALE = 1.0 / float(np.sqrt(HD))
NEG = -1.0e30
NT = S // 256               # 8 token chunks of 256
NK = H // 128               # 16 contraction tiles

_CACHE = {}


def _attn_meta(qi):
    qc = qi * 256
    jstart = max(0, qc // 128 - 8)
    jend = qc // 128 + 1
    js = list(range(jstart, jend + 1))
    if qc - js[0] * 128 == 1024:
        js[0], js[1] = js[1], js[0]
    return qc, js, jend


def _contrib(delta):
    # which 128-query chunks of the [128,512] pb tile are valid
    if delta == 1024:
        return (0, 2)          # first q-half of each head
    if delta == -128:
        return (1, 3)          # second q-half of each head
    return (0, 1, 2, 3)


def _build_program():
    import concourse.bacc as bacc
    import concourse.mybir as mybir
    import concourse.tile as tile

    f32 = mybir.dt.float32
    bf16 = mybir.dt.bfloat16
    Exp = mybir.ActivationFunctionType.Exp
    mult = mybir.AluOpType.mult
    add = mybir.AluOpType.add

    nc = bacc.Bacc("TRN2", target_bir_lowering=False, debug=False,
                   enable_asserts=False, num_devices=NCORES)

    hT_d = nc.dram_tensor("hT", [128, NT * 4096], bf16, kind="ExternalInput")
    wpk_d = nc.dram_tensor("wpk", [128, 4 * 4096], bf16, kind="ExternalInput")
    wo_d = nc.dram_tensor("wo", [512, 2048], bf16, kind="ExternalInput")
    cs_d = nc.dram_tensor("cs", [128, S], bf16, kind="ExternalInput")
    sn_d = nc.dram_tensor("sn", [128, S], bf16, kind="ExternalInput")
    cw_d = nc.dram_tensor("cw", [128, 8], f32, kind="ExternalInput")
    msk_d = nc.dram_tensor("msk", [128, 2048], bf16, kind="ExternalInput")
    ey6_d = nc.dram_tensor("ey6", [128, 128], bf16, kind="ExternalInput")
    one_d = nc.dram_tensor("one", [128, 8], bf16, kind="ExternalInput")
    yT_d = nc.dram_tensor("yT", [128, NT * 4096], bf16, kind="ExternalOutput")

    with tile.TileContext(nc) as tc:
        with (
            tc.tile_pool(name="const", bufs=1) as cp,
            tc.tile_pool(name="persist", bufs=1) as pp,
            tc.tile_pool(name="wfp", bufs=1) as wfp,
            tc.tile_pool(name="htp", bufs=2) as htp,
            tc.tile_pool(name="pbp", bufs=40) as pbp,
            tc.tile_pool(name="ep", bufs=4) as ep,
            tc.tile_pool(name="ssp", bufs=2) as ssp,
            tc.tile_pool(name="rbp", bufs=2) as rbp,
            tc.tile_pool(name="ybp", bufs=2) as ybp,
            tc.tile_pool(name="gp", bufs=3, space="PSUM") as gp,
            tc.tile_pool(name="scp", bufs=3, space="PSUM") as scp,
            tc.tile_pool(name="pvp", bufs=2, space="PSUM") as pvp,
        ):
            cs_sb = cp.tile([128, S], bf16, tag="cs", name="cs")
            sn_sb = cp.tile([128, S], bf16, tag="sn", name="sn")
            cw_sb = cp.tile([128, 8], f32, tag="cw", name="cw")
            ey6_sb = cp.tile([128, 128], bf16, tag="ey6", name="ey6")
            one_sb = cp.tile([128, 8], bf16, tag="one", name="one")
            msk_sb = cp.tile([128, 2048], bf16, tag="msk", name="msk")

            wf = [wfp.tile([128, 4096], bf16, tag=f"wf{i}", name=f"wf{i}")
                  for i in range(4)]
            qpair = [pp.tile([128, 2 * S], bf16, tag=f"qp{i}", name=f"qp{i}")
                     for i in range(KVH)]
            kconv = [pp.tile([128, S], bf16, tag=f"kc{i}", name=f"kc{i}")
                     for i in range(KVH)]
            kbuf = [pp.tile([128, 512], bf16, tag=f"kb{i}", name=f"kb{i}")
                    for i in range(KVH)]
            vbuf = [pp.tile([128, 512], bf16, tag=f"vb{i}", name=f"vb{i}")
                    for i in range(KVH)]
            vt = [[pp.tile([128, 128], bf16, tag=f"vt{i}_{j}", name=f"vt{i}_{j}")
                   for j in range(NK)] for i in range(KVH)]
            attn = [pp.tile([128, S], bf16, tag=f"at{h}", name=f"at{h}")
                    for h in range(QH)]
            wo_sb = [pp.tile([128, 2048], bf16, tag=f"wo{d}", name=f"wo{d}")
                     for d in range(QH)]

            # ---- startup DMA queue ------------------------------------
            hts = [None] * NT
            hts[0] = htp.tile([128, 4096], bf16, tag="ht", name="ht0")
            for qtr in range(4):
                sl = slice(qtr * 1024, (qtr + 1) * 1024)
                nc.sync.dma_start(out=hts[0][:, sl], in_=hT_d[:, sl])
                nc.sync.dma_start(out=wf[0][:, sl], in_=wpk_d[:, sl])
            nc.sync.dma_start(out=wf[1][:, 0:2048], in_=wpk_d[:, 4096:6144])
            nc.sync.dma_start(out=wf[1][:, 2048:4096], in_=wpk_d[:, 6144:8192])
            nc.sync.dma_start(out=cs_sb[:, 0:256], in_=cs_d[:, 0:256])
            nc.sync.dma_start(out=sn_sb[:, 0:256], in_=sn_d[:, 0:256])
            nc.sync.dma_start(out=wf[2][:, 0:2048], in_=wpk_d[:, 8192:10240])
            nc.sync.dma_start(out=wf[2][:, 2048:4096], in_=wpk_d[:, 10240:12288])
            nc.sync.dma_start(out=wf[3][:, 0:2048], in_=wpk_d[:, 12288:14336])
            nc.sync.dma_start(out=wf[3][:, 2048:4096], in_=wpk_d[:, 14336:16384])
            nc.sync.dma_start(out=cw_sb[:], in_=cw_d[:, :])
            nc.sync.dma_start(out=ey6_sb[:], in_=ey6_d[:, :])
            hts[1] = htp.tile([128, 4096], bf16, tag="ht", name="ht1")
            nc.sync.dma_start(out=hts[1][:, 0:2048], in_=hT_d[:, 4096:6144])
            nc.sync.dma_start(out=hts[1][:, 2048:4096], in_=hT_d[:, 6144:8192])
            nc.sync.dma_start(out=msk_sb[:], in_=msk_d[:, :])
            nc.sync.dma_start(out=cs_sb[:, 256:S], in_=cs_d[:, 256:S])
            nc.sync.dma_start(out=sn_sb[:, 256:S], in_=sn_d[:, 256:S])
            nc.sync.dma_start(out=one_sb[:], in_=one_d[:, :])
            for d in range(QH):
                nc.sync.dma_start(out=wo_sb[d][:],
                                  in_=wo_d[d * 128:(d + 1) * 128, :])

            # PE warm-up: matmuls on a never-written scratch tile ramp the
            # tensor engine to full clock while the first DMAs land
            warm = cp.tile([128, 256], bf16, tag="warm", name="warm")
            nc.vector.memset(warm[:], 0)
            wps = gp.tile([128, 512], f32, tag="g", name="warmps")
            for wk in range(26):
                nc.tensor.matmul(wps[:, 0:256], warm[:, 0:128], warm[:],
                                 start=(wk == 0), stop=(wk == 25))

            pb_store = {}

            def emit_score(qi, i, j):
                qc = qi * 256
                delta = qc - j * 128
                ps_sc = scp.tile([128, 512], f32, tag="sc", name="sc")
                lhs = kconv[i][:, j * 128:(j + 1) * 128]
                q2 = qpair[i][:].rearrange("p (h s) -> p h s", h=2)
                ps3 = ps_sc.rearrange("p (h q) -> p h q", h=2)
                if delta == 1024:
                    nc.tensor.matmul(ps3[:, :, 0:128], lhs,
                                     q2[:, :, qc:qc + 128],
                                     start=True, stop=True)
                elif delta == -128:
                    nc.tensor.matmul(ps3[:, :, 128:256], lhs,
                                     q2[:, :, qc + 128:qc + 256],
                                     start=True, stop=True)
                else:
                    nc.tensor.matmul(ps_sc[:], lhs, q2[:, :, qc:qc + 256],
                                     start=True, stop=True)
                # in-place PSUM mask add on the affected half only
                mt = {1024: 0, 896: 1, 0: 2, -128: 3}.get(delta)
                if mt is not None:
                    seg = slice(0, 128) if delta in (1024, 0) else slice(128, 256)
                    mk3 = msk_sb[:, mt * 512:(mt + 1) * 512].rearrange(
                        "p (h q) -> p h q", h=2)
                    nc.vector.tensor_add(ps3[:, :, seg], ps3[:, :, seg],
                                         mk3[:, :, seg])
                return (qi, i, j, delta, ps_sc, ps3)

            def emit_score_exp(sc_st):
                qi, i, j, delta, ps_sc, ps3 = sc_st
                pb = pbp.tile([128, 512], bf16, tag="pb", name="pb")
                pb3 = pb.rearrange("p (h q) -> p h q", h=2)
                if delta == 1024:
                    nc.scalar.activation(pb3[:, :, 0:128], ps3[:, :, 0:128],
                                         Exp, bias=0.0, scale=SCALE)
                elif delta == -128:
                    nc.scalar.activation(pb3[:, :, 128:256], ps3[:, :, 128:256],
                                         Exp, bias=0.0, scale=SCALE)
                else:
                    nc.scalar.activation(pb[:], ps_sc[:], Exp, bias=0.0,
                                         scale=SCALE)
                pb_store[(qi, i, j)] = pb

            def emit_pv(qi):
                qc, js, jend = _attn_meta(qi)
                ps_os = []
                ps_s = scp.tile([128, 8], f32, tag="sc", name="sms")
                for i in range(KVH):
                    if ps_os:
                        emit_norm_i(qi, 0, ps_os[0], ps_s)
                    ps_o = pvp.tile([128, 512], f32, tag="pv", name="pv")
                    ps_os.append(ps_o)
                    po3 = ps_o.rearrange("p (h q) -> p h q", h=2)
                    jfirst = js[0]
                    pbs = {}
                    for j in js:
                        delta = qc - j * 128
                        pb = pb_store.pop((qi, i, j))
                        pbs[j] = pb
                        pb3 = pb.rearrange("p (h q) -> p h q", h=2)
                        if delta == 1024:
                            nc.tensor.matmul(po3[:, :, 0:128], vt[i][j][:],
                                             pb3[:, :, 0:128],
                                             start=False, stop=False)
                        elif delta == -128:
                            nc.tensor.matmul(po3[:, :, 128:256], vt[i][j][:],
                                             pb3[:, :, 128:256],
                                             start=False, stop=True)
                        else:
                            nc.tensor.matmul(ps_o[:], vt[i][j][:], pb[:],
                                             start=(j == jfirst), stop=False)
                    # one accumulation chain at a time per bank: emit each
                    # sum column's chain contiguously
                    for c in range(4):
                        cjs = [j for j in js if c in _contrib(qc - j * 128)]
                        col = i * 4 + c
                        for j in cjs:
                            nc.tensor.matmul(
                                ps_s[:, col:col + 1],
                                pbs[j][:, c * 128:(c + 1) * 128],
                                one_sb[:, 0:1],
                                start=(j == cjs[0]), stop=(j == cjs[-1]))
                emit_norm_i(qi, 1, ps_os[1], ps_s)
                return ps_os, ps_s

            def emit_norm_i(qi, i, ps_o, ps_s):
                # per-kv-head normalization chain: reciprocal -> SBUF flatten
                # DMA -> partition broadcast -> 4 scaling multiplies
                qc = qi * 256
                rs4 = ssp.tile([128, 4], f32, tag="ss", name="rs4")
                nc.vector.reciprocal(rs4[:], ps_s[:, i * 4:(i + 1) * 4])
                rsf = ssp.tile([1, 512], f32, tag="rsf", name="rsf")
                nc.sync.dma_start(out=rsf[0:1, :], in_=rs4[:, :])
                rb = rbp.tile([128, 512], f32, tag="rb", name="rb")
                nc.gpsimd.partition_broadcast(rb[:], rsf[0:1, :])
                rbv = rb.rearrange("p (q c) -> p q c", c=4)
                for h in range(2):
                    for qh in range(2):
                        nc.vector.tensor_mul(
                            attn[2 * i + h][:, qc + qh * 128:
                                            qc + (qh + 1) * 128],
                            ps_o[:, h * 256 + qh * 128:
                                 h * 256 + (qh + 1) * 128],
                            rbv[:, :, 2 * h + qh])

            def emit_oproj_chain(u, oc, ybig, act_only=False):
                ps_y = gp.tile([128, 512], f32, tag="g", name="psy")
                for d in range(QH):
                    nc.tensor.matmul(
                        ps_y[:, 0:256],
                        wo_sb[d][:, oc * 128:(oc + 1) * 128],
                        attn[d][:, u * 256:(u + 1) * 256],
                        start=(d == 0), stop=(d == QH - 1))
                dst = ybig[:, oc * 256:(oc + 1) * 256]
                if oc % 2 == 0 and not act_only:
                    nc.vector.tensor_copy(dst, ps_y[:, 0:256])
                else:
                    nc.scalar.copy(dst, ps_y[:, 0:256])

            def emit_qkv_col(t, col, ps_half):
                # rope / conv / v processing for one 128-dim output column
                cur, prv = (t % 2) * 256, ((t + 1) % 2) * 256
                csl = cs_sb[:, t * 256:(t + 1) * 256]
                snl = sn_sb[:, t * 256:(t + 1) * 256]
                if col < 6:
                    # stage the PSUM column into SBUF bf16 on the Activation
                    # engine (releases the accumulator early), then run the
                    # rope multiplies at the DVE 2x 16-bit rate. e2 is built
                    # pre-shifted (e2[p] = ps[(p+64)%128]*sn[p]), exploiting
                    # sn's identical halves, so the final sub/add read SBUF
                    # operands with matching base partitions.
                    psc = ep.tile([128, 256], bf16, tag="psc", name="psc")
                    nc.scalar.copy(psc[:], ps_half)
                    pscr = ep.tile([128, 256], bf16, tag="pscr", name="pscr")
                    nc.vector.tensor_copy(pscr[0:64, :], psc[64:128, :])
                    nc.vector.tensor_copy(pscr[64:128, :], psc[0:64, :])
                    e1 = ep.tile([128, 256], bf16, tag="e1", name="e1")
                    e2 = ep.tile([128, 256], bf16, tag="e2", name="e2")
                    nc.vector.tensor_mul(e1[:], psc[:], csl)
                    nc.vector.tensor_mul(e2[:], pscr[:], snl)
                    if col < 4:
                        dest = qpair[col // 2]
                        off = (col % 2) * S + t * 256
                    else:
                        dest = kbuf[col - 4]
                        off = cur
                    nc.vector.tensor_sub(dest[0:64, off:off + 256],
                                         e1[0:64, :], e2[0:64, :])
                    nc.vector.tensor_add(dest[64:128, off:off + 256],
                                         e2[64:128, :], e1[64:128, :])
                    if col in (4, 5):
                        i = col - 4
                        w0k = cw_sb[:, 2 * i:2 * i + 1]
                        w1k = cw_sb[:, 2 * i + 1:2 * i + 2]
                        kc = kconv[i]
                        tmp = ep.tile([128, 256], bf16, tag="ct", name="ct")
                        nc.vector.tensor_scalar_mul(tmp[:],
                                                    kbuf[i][:, cur:cur + 256],
                                                    w1k)
                        nc.vector.scalar_tensor_tensor(
                            kc[:, t * 256 + 1:t * 256 + 256],
                            kbuf[i][:, cur:cur + 255], w0k, tmp[:, 1:256],
                            mult, add)
                        if t == 0:
                            nc.vector.tensor_copy(kc[:, 0:1], tmp[:, 0:1])
                        else:
                            nc.vector.scalar_tensor_tensor(
                                kc[:, t * 256:t * 256 + 1],
                                kbuf[i][:, prv + 255:prv + 256], w0k,
                                tmp[:, 0:1], mult, add)
                else:
                    i = col - 6
                    w0v = cw_sb[:, 4 + 2 * i:5 + 2 * i]
                    w1v = cw_sb[:, 5 + 2 * i:6 + 2 * i]
                    nc.scalar.copy(vbuf[i][:, cur:cur + 256], ps_half)
                    vcb = ep.tile([128, 256], bf16, tag="vcb", name="vcb")
                    tm2 = ep.tile([128, 256], bf16, tag="ct2", name="ct2")
                    nc.vector.tensor_scalar_mul(tm2[:],
                                                vbuf[i][:, cur:cur + 256], w1v)
                    nc.vector.scalar_tensor_tensor(
                        vcb[:, 1:256], vbuf[i][:, cur:cur + 255], w0v,
                        tm2[:, 1:256], mult, add)
                    if t == 0:
                        nc.vector.tensor_copy(vcb[:, 0:1], tm2[:, 0:1])
                    else:
                        nc.vector.scalar_tensor_tensor(
                            vcb[:, 0:1], vbuf[i][:, prv + 255:prv + 256], w0v,
                            tm2[:, 0:1], mult, add)
                    for hh in range(2):
                        tp_ = gp.tile([128, 128], bf16, tag="g", name="vtp")
                        nc.tensor.transpose(tp_[:],
                                            vcb[:, hh * 128:(hh + 1) * 128],
                                            ey6_sb[:])
                        if hh == 0:
                            nc.scalar.copy(vt[i][2 * t + hh][:], tp_[:])
                        else:
                            nc.vector.tensor_copy(vt[i][2 * t + hh][:], tp_[:])

            # ---- main software-pipelined loop -------------------------
            for t in range(NT):
                if 0 < t < NT - 1:
                    ht_n = htp.tile([128, 4096], bf16, tag="ht",
                                    name=f"ht{t + 1}")
                    nc.sync.dma_start(
                        out=ht_n[:],
                        in_=hT_d[:, (t + 1) * 4096:(t + 2) * 4096])
                    hts[t + 1] = ht_n

                units = []
                if t >= 1:
                    qi = t - 1
                    _, js, _ = _attn_meta(qi)
                    units = [(qi, i, j) for i in range(KVH) for j in js]
                # split score units into 4 groups, one after each col pair
                ugroups = [units[(len(units) * g) // 4:
                                 (len(units) * (g + 1)) // 4] for g in range(4)]
                if t == NT - 1:
                    # kconv(t)/qpair(t) are complete after cp2: start this
                    # chunk's own scores early so the tail isn't exp-bound
                    _, js7, _ = _attn_meta(t)
                    u7 = [(t, i, j) for i in range(KVH) for j in js7]
                    ugroups[3] = ugroups[3] + u7[:len(u7) // 2]

                uq = [u for grp in ugroups for u in grp]
                un = 0
                for cpi in range(4):
                    ht = hts[t]
                    pss = []
                    pend = []
                    for half in range(2):
                        ps = gp.tile([128, 512], f32, tag="g", name="qkvps")
                        for k in range(NK):
                            nc.tensor.matmul(
                                ps[:, 0:256],
                                wf[cpi][:, k * 256 + half * 128:
                                        k * 256 + half * 128 + 128],
                                ht[:, k * 256:(k + 1) * 256],
                                start=(k == 0), stop=(k == NK - 1))
                            # pace score units at roughly the exp rate; exps
                            # are deferred past the psc copies so the QKV
                            # accumulators release without queueing on Act
                            if k in (5, 11) and un < len(uq):
                                emit_score_exp(emit_score(*uq[un]))
                                un += 1
                        pss.append(ps)
                    emit_qkv_col(t, 2 * cpi, pss[0][:, 0:256])
                    emit_qkv_col(t, 2 * cpi + 1, pss[1][:, 0:256])
                    while un < (len(uq) * (cpi + 1)) // 4:
                        emit_score_exp(emit_score(*uq[un]))
                        un += 1

                if t >= 1:
                    emit_pv(t - 1)
                if t >= 2:
                    u = t - 2
                    ybig = ybp.tile([128, 4096], bf16, tag="yb", name="yb")
                    for oc in range(NK):
                        emit_oproj_chain(u, oc, ybig)
                        if oc % 4 == 3:
                            sl = slice((oc - 3) * 256, (oc + 1) * 256)
                            nc.sync.dma_start(
                                out=yT_d[:, u * 4096 + sl.start:
                                         u * 4096 + sl.stop],
                                in_=ybig[:, sl])

            # ---- tail: attn(7) interleaved with o_proj(6), then o_proj(7)
            qi = NT - 1
            units = u7[len(u7) // 2:]
            ybig6 = ybp.tile([128, 4096], bf16, tag="yb", name="yb6")
            ui = 0
            for oc in range(6):
                take = ((oc + 1) * len(units)) // 6 - (oc * len(units)) // 6
                for _ in range(take):
                    emit_score_exp(emit_score(*units[ui]))
                    ui += 1
                emit_oproj_chain(NT - 2, oc, ybig6)
                if oc % 4 == 3:
                    sl = slice((oc - 3) * 256, (oc + 1) * 256)
                    nc.sync.dma_start(
                        out=yT_d[:, (NT - 2) * 4096 + sl.start:
                                 (NT - 2) * 4096 + sl.stop],
                        in_=ybig6[:, sl])
            emit_pv(qi)
            # last 10 o_proj(6) chains fill the norm(7) chain latency;
            # copies go to Act so DVE can run the norm multiplies at once
            for oc in range(6, NK):
                emit_oproj_chain(NT - 2, oc, ybig6, act_only=True)
                if oc % 4 == 3:
                    sl = slice((oc - 3) * 256, (oc + 1) * 256)
                    nc.sync.dma_start(
                        out=yT_d[:, (NT - 2) * 4096 + sl.start:
                                 (NT - 2) * 4096 + sl.stop],
                        in_=ybig6[:, sl])
            ybig7 = ybp.tile([128, 4096], bf16, tag="yb", name="yb7")
            for oc in range(NK):
                emit_oproj_chain(NT - 1, oc, ybig7)
                if oc % 2 == 1:
                    sl = slice((oc - 1) * 256, (oc + 1) * 256)
                    nc.sync.dma_start(
                        out=yT_d[:, (NT - 1) * 4096 + sl.start:
                                 (NT - 1) * 4096 + sl.stop],
                        in_=ybig7[:, sl])

    nc.finalize()
    return nc


def _host_inputs(hidden, W_pack, W_o, conv_k, conv_v):
    """Per-core input maps (all bf16, host-blocked layouts)."""
    bf = ml_dtypes.bfloat16
    pos = np.arange(S, dtype=np.float64)
    inv_freq = 1.0 / (THETA ** (np.arange(0, HD, 2, dtype=np.float64) / HD))
    freqs = np.outer(pos, inv_freq)                       # (S, 64)
    cos = np.cos(freqs).T.astype(np.float32)              # (64, S)
    sin = np.sin(freqs).T.astype(np.float32)
    cs = np.concatenate([cos, cos], axis=0).astype(bf)    # (128, S)
    sn = np.concatenate([sin, sin], axis=0).astype(bf)

    kk = np.arange(128)[:, None]
    qq = np.arange(256)[None, :]

    def double(m):
        return np.concatenate([m, m], axis=1).astype(np.float32)

    t0 = double(np.where(kk <= qq, 0.0, NEG))             # delta = 0
    tm128 = double(np.where(kk <= qq - 128, 0.0, NEG))    # delta = -128
    w896 = double(np.where(qq - kk < 128, 0.0, NEG))      # delta = 896
    w1024 = double(np.where(qq < kk, 0.0, NEG))           # delta = 1024
    msk = np.concatenate([w1024, w896, t0, tm128], axis=1).astype(bf)

    ey6 = np.eye(128, dtype=np.float32).astype(bf)
    one = np.ones((128, 8), dtype=np.float32).astype(bf)

    in_maps = []
    for c in range(NCORES):
        b, g = c // TP, c % TP
        # hidden chunk-k blocked: [p, t*4096 + k*256 + tok]
        hblk = np.ascontiguousarray(
            hidden[b].astype(bf).reshape(NT, 256, NK, 128)
            .transpose(3, 0, 2, 1).reshape(128, NT * 4096))
        wq = W_pack[:, g * 512:(g + 1) * 512]
        wk = W_pack[:, NH * HD + 2 * g * 128: NH * HD + (2 * g + 2) * 128]
        wv = W_pack[:, NH * HD + NKV * HD + 2 * g * 128:
                    NH * HD + NKV * HD + (2 * g + 2) * 128]
        wsel = np.concatenate([wq, wk, wv], axis=1).astype(bf)  # (2048, 1024)
        # col-pair-k blocked: [p, cp*4096 + k*256 + cc]
        wblk = np.ascontiguousarray(
            wsel.reshape(NK, 128, 4, 256).transpose(1, 2, 0, 3)
            .reshape(128, 4 * 4096))
        wo = np.ascontiguousarray(
            W_o[g * 512:(g + 1) * 512, :]).astype(bf)
        cwv = np.empty(8, np.float32)
        for i in range(KVH):
            cwv[2 * i] = conv_k[2 * g + i, 0]
            cwv[2 * i + 1] = conv_k[2 * g + i, 1]
            cwv[4 + 2 * i] = conv_v[2 * g + i, 0]
            cwv[4 + 2 * i + 1] = conv_v[2 * g + i, 1]
        cw = np.broadcast_to(cwv, (128, 8)).astype(np.float32).copy()
        in_maps.append({
            "hT": hblk, "wpk": wblk, "wo": wo, "cs": cs, "sn": sn,
            "cw": cw, "msk": msk, "ey6": ey6, "one": one,
        })
    return in_maps


def run_cores(in_maps, trace=False, **kw):
    from concourse.bass_utils import run_bass_kernel_spmd
    if "nc" not in _CACHE:
        _CACHE["nc"] = _build_program()
    return run_bass_kernel_spmd(_CACHE["nc"], in_maps, list(range(NCORES)),
                                trace=trace, **kw)


def kernel(hidden, W_pack, W_o, conv_k, conv_v):
    hidden = np.asarray(hidden, np.float32)
    W_pack = np.asarray(W_pack, np.float32)
    W_o = np.asarray(W_o, np.float32)
    conv_k = np.asarray(conv_k, np.float32)
    conv_v = np.asarray(conv_v, np.float32)
    in_maps = _host_inputs(hidden, W_pack, W_o, conv_k, conv_v)
    res = run_cores(in_maps)
    out = np.zeros((B, S, H), np.float32)
    for c in range(NCORES):
        b = c // TP
        # yT blocked [p, u*4096 + oc*256 + tok] -> partial [H, S]
        arr = np.asarray(res.results[c]["yT"]).astype(np.float32)
        part = arr.reshape(128, NT, NK, 256).transpose(2, 0, 1, 3).reshape(H, S)
        out[b] += part.T
    return out


# revision 7
# speedup vs baseline: 1.0005x; 1.0005x over previous
"""Baichuan sliding-window GQA attention block on 8 trn2 NeuronCores.

Sharding: data-parallel over batch (2) x tensor-parallel over heads (4).
Core c handles batch b=c//4, head group g=c%4 (q heads 4g..4g+3, kv heads
2g..2g+1). Each core computes the fused qkv projection, RoPE, 2-tap causal
conv, windowed attention and a row-sharded o_proj partial; the host sums
the 4 partials per batch.

Performance structure (295us -> 229us vs the fp32 v1 kernel):
  - all SBUF/DRAM storage is bf16 (PSUM accumulation stays fp32): halves
    DMA traffic and runs packed DVE ops at 2-4x rate. Matmuls keep the
    1 cycle/row PE rate they had with f32r.
  - host-blocked DRAM layouts (hidden chunk-k-blocked, W_pack col-pair-k
    blocked, output u-oc-blocked) so every transfer is one large DMA;
    the HWDGE fixed overhead (~625ns/DMA) makes small DMAs expensive.
  - softmax denominators come from stationary-pb matmuls (pb chunk as the
    128-wide stationary operand, ones as a 1-row moving operand), which
    cost ~1 PE cycle each instead of streaming 512 rows per ones-matmul.
    The [128q, col] sums are reciprocated, flattened to partition 0 by a
    tiny SBUF->SBUF DMA, partition-broadcast on GPSIMD, and applied as
    per-128-query scaling multiplies with stride-8 access patterns.
  - one software-pipelined loop: QKV(t) chains carry the scores(t-1)
    matmuls between their contraction steps (paced at the Activation
    engine's exp rate), followed by PV(t-1)+sums, the per-kv-head
    normalization chains, and o_proj(t-2). scores(7) start inside
    iteration 7 and the tail interleaves o_proj(6) around PV(7) so the
    exp drain and the normalization latency stay off the critical path.
  - masks are added in-place into the scores PSUM only on the affected
    half-tiles; window-edge tiles compute/exp only their valid halves.
  - one accumulation chain per PSUM bank at a time: interleaving chains
    in a bank silently corrupts earlier chains' partials on hardware
    (verified empirically), so qkv columns and sum columns each run as
    contiguous chains.
  - a short warm-up matmul chain on scratch data ramps the PE out of its
    low p-state while the first weight DMAs land.
"""

import numpy as np
import ml_dtypes

B, S, H = 2, 2048, 2048
NH, NKV, HD = 16, 8, 128
WINDOW = 1024
THETA = 100000.0
TP = 4                      # tensor-parallel ways (head groups)
QH = NH // TP               # 4 q heads per core
KVH = NKV // TP             # 2 kv heads per core
NCORES = 8
SCALE = 1.0 / float(np.sqrt(HD))
NEG = -1.0e30
NT = S // 256               # 8 token chunks of 256
NK = H // 128               # 16 contraction tiles

_CACHE = {}


def _attn_meta(qi):
    qc = qi * 256
    jstart = max(0, qc // 128 - 8)
    jend = qc // 128 + 1
    js = list(range(jstart, jend + 1))
    if qc - js[0] * 128 == 1024:
        js[0], js[1] = js[1], js[0]
    return qc, js, jend


def _contrib(delta):
    # which 128-query chunks of the [128,512] pb tile are valid
    if delta == 1024:
        return (0, 2)          # first q-half of each head
    if delta == -128:
        return (1, 3)          # second q-half of each head
    return (0, 1, 2, 3)


def _build_program():
    import concourse.bacc as bacc
    import concourse.mybir as mybir
    import concourse.tile as tile

    f32 = mybir.dt.float32
    bf16 = mybir.dt.bfloat16
    Exp = mybir.ActivationFunctionType.Exp
    mult = mybir.AluOpType.mult
    add = mybir.AluOpType.add

    nc = bacc.Bacc("TRN2", target_bir_lowering=False, debug=False,
                   enable_asserts=False, num_devices=NCORES)

    hT_d = nc.dram_tensor("hT", [128, NT * 4096], bf16, kind="ExternalInput")
    wpk_d = nc.dram_tensor("wpk", [128, 4 * 4096], bf16, kind="ExternalInput")
    wo_d = nc.dram_tensor("wo", [512, 2048], bf16, kind="ExternalInput")
    cs_d = nc.dram_tensor("cs", [128, S], bf16, kind="ExternalInput")
    sn_d = nc.dram_tensor("sn", [128, S], bf16, kind="ExternalInput")
    cw_d = nc.dram_tensor("cw", [128, 8], f32, kind="ExternalInput")
    msk_d = nc.dram_tensor("msk", [128, 2048], bf16, kind="ExternalInput")
    ey6_d = nc.dram_tensor("ey6", [128, 128], bf16, kind="ExternalInput")
    one_d = nc.dram_tensor("one", [128, 8], bf16, kind="ExternalInput")
    yT_d = nc.dram_tensor("yT", [128, NT * 4096], bf16, kind="ExternalOutput")

    with tile.TileContext(nc) as tc:
        with (
            tc.tile_pool(name="const", bufs=1) as cp,
            tc.tile_pool(name="persist", bufs=1) as pp,
            tc.tile_pool(name="wfp", bufs=1) as wfp,
            tc.tile_pool(name="htp", bufs=2) as htp,
            tc.tile_pool(name="pbp", bufs=40) as pbp,
            tc.tile_pool(name="ep", bufs=4) as ep,
            tc.tile_pool(name="ssp", bufs=2) as ssp,
            tc.tile_pool(name="rbp", bufs=2) as rbp,
            tc.tile_pool(name="ybp", bufs=2) as ybp,
            tc.tile_pool(name="gp", bufs=3, space="PSUM") as gp,
            tc.tile_pool(name="scp", bufs=3, space="PSUM") as scp,
            tc.tile_pool(name="pvp", bufs=2, space="PSUM") as pvp,
        ):
            cs_sb = cp.tile([128, S], bf16, tag="cs", name="cs")
            sn_sb = cp.tile([128, S], bf16, tag="sn", name="sn")
            cw_sb = cp.tile([128, 8], f32, tag="cw", name="cw")
            ey6_sb = cp.tile([128, 128], bf16, tag="ey6", name="ey6")
            one_sb = cp.tile([128, 8], bf16, tag="one", name="one")
            msk_sb = cp.tile([128, 2048], bf16, tag="msk", name="msk")

            wf = [wfp.tile([128, 4096], bf16, tag=f"wf{i}", name=f"wf{i}")
                  for i in range(4)]
            qpair = [pp.tile([128, 2 * S], bf16, tag=f"qp{i}", name=f"qp{i}")
                     for i in range(KVH)]
            kconv = [pp.tile([128, S], bf16, tag=f"kc{i}", name=f"kc{i}")
                     for i in range(KVH)]
            kbuf = [pp.tile([128, 512], bf16, tag=f"kb{i}", name=f"kb{i}")
                    for i in range(KVH)]
            vbuf = [pp.tile([128, 512], bf16, tag=f"vb{i}", name=f"vb{i}")
                    for i in range(KVH)]
            vt = [[pp.tile([128, 128], bf16, tag=f"vt{i}_{j}", name=f"vt{i}_{j}")
                   for j in range(NK)] for i in range(KVH)]
            attn = [pp.tile([128, S], bf16, tag=f"at{h}", name=f"at{h}")
                    for h in range(QH)]
            wo_sb = [pp.tile([128, 2048], bf16, tag=f"wo{d}", name=f"wo{d}")
                     for d in range(QH)]

            # ---- startup DMA queue ------------------------------------
            hts = [None] * NT
            hts[0] = htp.tile([128, 4096], bf16, tag="ht", name="ht0")
            for qtr in range(4):
                sl = slice(qtr * 1024, (qtr + 1) * 1024)
                nc.sync.dma_start(out=hts[0][:, sl], in_=hT_d[:, sl])
                nc.sync.dma_start(out=wf[0][:, sl], in_=wpk_d[:, sl])
            nc.sync.dma_start(out=wf[1][:, 0:2048], in_=wpk_d[:, 4096:6144])
            nc.sync.dma_start(out=wf[1][:, 2048:4096], in_=wpk_d[:, 6144:8192])
            nc.sync.dma_start(out=cs_sb[:, 0:256], in_=cs_d[:, 0:256])
            nc.sync.dma_start(out=sn_sb[:, 0:256], in_=sn_d[:, 0:256])
            nc.sync.dma_start(out=wf[2][:, 0:2048], in_=wpk_d[:, 8192:10240])
            nc.sync.dma_start(out=wf[2][:, 2048:4096], in_=wpk_d[:, 10240:12288])
            nc.sync.dma_start(out=wf[3][:, 0:2048], in_=wpk_d[:, 12288:14336])
            nc.sync.dma_start(out=wf[3][:, 2048:4096], in_=wpk_d[:, 14336:16384])
            nc.sync.dma_start(out=cw_sb[:], in_=cw_d[:, :])
            nc.sync.dma_start(out=ey6_sb[:], in_=ey6_d[:, :])
            hts[1] = htp.tile([128, 4096], bf16, tag="ht", name="ht1")
            nc.sync.dma_start(out=hts[1][:, 0:2048], in_=hT_d[:, 4096:6144])
            nc.sync.dma_start(out=hts[1][:, 2048:4096], in_=hT_d[:, 6144:8192])
            nc.sync.dma_start(out=msk_sb[:], in_=msk_d[:, :])
            nc.sync.dma_start(out=cs_sb[:, 256:S], in_=cs_d[:, 256:S])
            nc.sync.dma_start(out=sn_sb[:, 256:S], in_=sn_d[:, 256:S])
            nc.sync.dma_start(out=one_sb[:], in_=one_d[:, :])
            for d in range(QH):
                nc.sync.dma_start(out=wo_sb[d][:],
                                  in_=wo_d[d * 128:(d + 1) * 128, :])

            # PE warm-up: matmuls on a never-written scratch tile ramp the
            # tensor engine to full clock while the first DMAs land
            warm = cp.tile([128, 256], bf16, tag="warm", name="warm")
            nc.vector.memset(warm[:], 0)
            wps = gp.tile([128, 512], f32, tag="g", name="warmps")
            for wk in range(10):
                nc.tensor.matmul(wps[:, 0:256], warm[:, 0:128], warm[:],
                                 start=(wk == 0), stop=(wk == 9))

            pb_store = {}

            def emit_score(qi, i, j):
                qc = qi * 256
                delta = qc - j * 128
                ps_sc = scp.tile([128, 512], f32, tag="sc", name="sc")
                lhs = kconv[i][:, j * 128:(j + 1) * 128]
                q2 = qpair[i][:].rearrange("p (h s) -> p h s", h=2)
                ps3 = ps_sc.rearrange("p (h q) -> p h q", h=2)
                if delta == 1024:
                    nc.tensor.matmul(ps3[:, :, 0:128], lhs,
                                     q2[:, :, qc:qc + 128],
                                     start=True, stop=True)
                elif delta == -128:
                    nc.tensor.matmul(ps3[:, :, 128:256], lhs,
                                     q2[:, :, qc + 128:qc + 256],
                                     start=True, stop=True)
                else:
                    nc.tensor.matmul(ps_sc[:], lhs, q2[:, :, qc:qc + 256],
                                     start=True, stop=True)
                # in-place PSUM mask add on the affected half only
                mt = {1024: 0, 896: 1, 0: 2, -128: 3}.get(delta)
                if mt is not None:
                    seg = slice(0, 128) if delta in (1024, 0) else slice(128, 256)
                    mk3 = msk_sb[:, mt * 512:(mt + 1) * 512].rearrange(
                        "p (h q) -> p h q", h=2)
                    nc.vector.tensor_add(ps3[:, :, seg], ps3[:, :, seg],
                                         mk3[:, :, seg])
                return (qi, i, j, delta, ps_sc, ps3)

            def emit_score_exp(sc_st):
                qi, i, j, delta, ps_sc, ps3 = sc_st
                pb = pbp.tile([128, 512], bf16, tag="pb", name="pb")
                pb3 = pb.rearrange("p (h q) -> p h q", h=2)
                if delta == 1024:
                    nc.scalar.activation(pb3[:, :, 0:128], ps3[:, :, 0:128],
                                         Exp, bias=0.0, scale=SCALE)
                elif delta == -128:
                    nc.scalar.activation(pb3[:, :, 128:256], ps3[:, :, 128:256],
                                         Exp, bias=0.0, scale=SCALE)
                else:
                    nc.scalar.activation(pb[:], ps_sc[:], Exp, bias=0.0,
                                         scale=SCALE)
                pb_store[(qi, i, j)] = pb

            def emit_pv(qi):
                qc, js, jend = _attn_meta(qi)
                ps_os = []
                ps_s = scp.tile([128, 8], f32, tag="sc", name="sms")
                for i in range(KVH):
                    if ps_os:
                        emit_norm_i(qi, 0, ps_os[0], ps_s)
                    ps_o = pvp.tile([128, 512], f32, tag="pv", name="pv")
                    ps_os.append(ps_o)
                    po3 = ps_o.rearrange("p (h q) -> p h q", h=2)
                    jfirst = js[0]
                    pbs = {}
                    for j in js:
                        delta = qc - j * 128
                        pb = pb_store.pop((qi, i, j))
                        pbs[j] = pb
                        pb3 = pb.rearrange("p (h q) -> p h q", h=2)
                        if delta == 1024:
                            nc.tensor.matmul(po3[:, :, 0:128], vt[i][j][:],
                                             pb3[:, :, 0:128],
                                             start=False, stop=False)
                        elif delta == -128:
                            nc.tensor.matmul(po3[:, :, 128:256], vt[i][j][:],
                                             pb3[:, :, 128:256],
                                             start=False, stop=True)
                        else:
                            nc.tensor.matmul(ps_o[:], vt[i][j][:], pb[:],
                                             start=(j == jfirst), stop=False)
                    # one accumulation chain at a time per bank: emit each
                    # sum column's chain contiguously
                    for c in range(4):
                        cjs = [j for j in js if c in _contrib(qc - j * 128)]
                        col = i * 4 + c
                        for j in cjs:
                            nc.tensor.matmul(
                                ps_s[:, col:col + 1],
                                pbs[j][:, c * 128:(c + 1) * 128],
                                one_sb[:, 0:1],
                                start=(j == cjs[0]), stop=(j == cjs[-1]))
                emit_norm_i(qi, 1, ps_os[1], ps_s)
                return ps_os, ps_s

            def emit_norm_i(qi, i, ps_o, ps_s):
                # per-kv-head normalization chain: reciprocal -> SBUF flatten
                # DMA -> partition broadcast -> 4 scaling multiplies
                qc = qi * 256
                rs4 = ssp.tile([128, 4], f32, tag="ss", name="rs4")
                nc.vector.reciprocal(rs4[:], ps_s[:, i * 4:(i + 1) * 4])
                rsf = ssp.tile([1, 512], f32, tag="rsf", name="rsf")
                nc.sync.dma_start(out=rsf[0:1, :], in_=rs4[:, :])
                rb = rbp.tile([128, 512], f32, tag="rb", name="rb")
                nc.gpsimd.partition_broadcast(rb[:], rsf[0:1, :])
                rbv = rb.rearrange("p (q c) -> p q c", c=4)
                for h in range(2):
                    for qh in range(2):
                        nc.vector.tensor_mul(
                            attn[2 * i + h][:, qc + qh * 128:
                                            qc + (qh + 1) * 128],
                            ps_o[:, h * 256 + qh * 128:
                                 h * 256 + (qh + 1) * 128],
                            rbv[:, :, 2 * h + qh])

            def emit_oproj_chain(u, oc, ybig, act_only=False):
                ps_y = gp.tile([128, 512], f32, tag="g", name="psy")
                for d in range(QH):
                    nc.tensor.matmul(
                        ps_y[:, 0:256],
                        wo_sb[d][:, oc * 128:(oc + 1) * 128],
                        attn[d][:, u * 256:(u + 1) * 256],
                        start=(d == 0), stop=(d == QH - 1))
                dst = ybig[:, oc * 256:(oc + 1) * 256]
                if oc % 2 == 0 and not act_only:
                    nc.vector.tensor_copy(dst, ps_y[:, 0:256])
                else:
                    nc.scalar.copy(dst, ps_y[:, 0:256])

            def emit_qkv_col(t, col, ps_half):
                # rope / conv / v processing for one 128-dim output column
                cur, prv = (t % 2) * 256, ((t + 1) % 2) * 256
                csl = cs_sb[:, t * 256:(t + 1) * 256]
                snl = sn_sb[:, t * 256:(t + 1) * 256]
                if col < 6:
                    # stage the PSUM column into SBUF bf16 on the Activation
                    # engine (releases the accumulator early), then run the
                    # rope multiplies at the DVE 2x 16-bit rate. e2 is built
                    # pre-shifted (e2[p] = ps[(p+64)%128]*sn[p]), exploiting
                    # sn's identical halves, so the final sub/add read SBUF
                    # operands with matching base partitions.
                    psc = ep.tile([128, 256], bf16, tag="psc", name="psc")
                    nc.scalar.copy(psc[:], ps_half)
                    pscr = ep.tile([128, 256], bf16, tag="pscr", name="pscr")
                    nc.vector.tensor_copy(pscr[0:64, :], psc[64:128, :])
                    nc.vector.tensor_copy(pscr[64:128, :], psc[0:64, :])
                    e1 = ep.tile([128, 256], bf16, tag="e1", name="e1")
                    e2 = ep.tile([128, 256], bf16, tag="e2", name="e2")
                    nc.vector.tensor_mul(e1[:], psc[:], csl)
                    nc.vector.tensor_mul(e2[:], pscr[:], snl)
                    if col < 4:
                        dest = qpair[col // 2]
                        off = (col % 2) * S + t * 256
                    else:
                        dest = kbuf[col - 4]
                        off = cur
                    nc.vector.tensor_sub(dest[0:64, off:off + 256],
                                         e1[0:64, :], e2[0:64, :])
                    nc.vector.tensor_add(dest[64:128, off:off + 256],
                                         e2[64:128, :], e1[64:128, :])
                    if col in (4, 5):
                        i = col - 4
                        w0k = cw_sb[:, 2 * i:2 * i + 1]
                        w1k = cw_sb[:, 2 * i + 1:2 * i + 2]
                        kc = kconv[i]
                        tmp = ep.tile([128, 256], bf16, tag="ct", name="ct")
                        nc.vector.tensor_scalar_mul(tmp[:],
                                                    kbuf[i][:, cur:cur + 256],
                                                    w1k)
                        nc.vector.scalar_tensor_tensor(
                            kc[:, t * 256 + 1:t * 256 + 256],
                            kbuf[i][:, cur:cur + 255], w0k, tmp[:, 1:256],
                            mult, add)
                        if t == 0:
                            nc.vector.tensor_copy(kc[:, 0:1], tmp[:, 0:1])
                        else:
                            nc.vector.scalar_tensor_tensor(
                                kc[:, t * 256:t * 256 + 1],
                                kbuf[i][:, prv + 255:prv + 256], w0k,
                                tmp[:, 0:1], mult, add)
                else:
                    i = col - 6
                    w0v = cw_sb[:, 4 + 2 * i:5 + 2 * i]
                    w1v = cw_sb[:, 5 + 2 * i:6 + 2 * i]
                    nc.scalar.copy(vbuf[i][:, cur:cur + 256], ps_half)
                    vcb = ep.tile([128, 256], bf16, tag="vcb", name="vcb")
                    tm2 = ep.tile([128, 256], bf16, tag="ct2", name="ct2")
                    nc.vector.tensor_scalar_mul(tm2[:],
                                                vbuf[i][:, cur:cur + 256], w1v)
                    nc.vector.scalar_tensor_tensor(
                        vcb[:, 1:256], vbuf[i][:, cur:cur + 255], w0v,
                        tm2[:, 1:256], mult, add)
                    if t == 0:
                        nc.vector.tensor_copy(vcb[:, 0:1], tm2[:, 0:1])
                    else:
                        nc.vector.scalar_tensor_tensor(
                            vcb[:, 0:1], vbuf[i][:, prv + 255:prv + 256], w0v,
                            tm2[:, 0:1], mult, add)
                    for hh in range(2):
                        tp_ = gp.tile([128, 128], bf16, tag="g", name="vtp")
                        nc.tensor.transpose(tp_[:],
                                            vcb[:, hh * 128:(hh + 1) * 128],
                                            ey6_sb[:])
                        if hh == 0:
                            nc.scalar.copy(vt[i][2 * t + hh][:], tp_[:])
                        else:
                            nc.vector.tensor_copy(vt[i][2 * t + hh][:], tp_[:])

            # ---- main software-pipelined loop -------------------------
            for t in range(NT):
                if 0 < t < NT - 1:
                    ht_n = htp.tile([128, 4096], bf16, tag="ht",
                                    name=f"ht{t + 1}")
                    nc.sync.dma_start(
                        out=ht_n[:],
                        in_=hT_d[:, (t + 1) * 4096:(t + 2) * 4096])
                    hts[t + 1] = ht_n

                units = []
                if t >= 1:
                    qi = t - 1
                    _, js, _ = _attn_meta(qi)
                    units = [(qi, i, j) for i in range(KVH) for j in js]
                # split score units into 4 groups, one after each col pair
                ugroups = [units[(len(units) * g) // 4:
                                 (len(units) * (g + 1)) // 4] for g in range(4)]
                if t == NT - 1:
                    # kconv(t)/qpair(t) are complete after cp2: start this
                    # chunk's own scores early so the tail isn't exp-bound
                    _, js7, _ = _attn_meta(t)
                    u7 = [(t, i, j) for i in range(KVH) for j in js7]
                    ugroups[3] = ugroups[3] + u7[:len(u7) // 2]

                uq = [u for grp in ugroups for u in grp]
                un = 0
                for cpn, cpi in enumerate(range(4)):
                    ht = hts[t]
                    pss = []
                    pend = []
                    for half in range(2):
                        ps = gp.tile([128, 512], f32, tag="g", name="qkvps")
                        for k in range(NK):
                            nc.tensor.matmul(
                                ps[:, 0:256],
                                wf[cpi][:, k * 256 + half * 128:
                                        k * 256 + half * 128 + 128],
                                ht[:, k * 256:(k + 1) * 256],
                                start=(k == 0), stop=(k == NK - 1))
                            # pace score units at roughly the exp rate; exps
                            # are deferred past the psc copies so the QKV
                            # accumulators release without queueing on Act
                            if k in (5, 11) and un < len(uq):
                                emit_score_exp(emit_score(*uq[un]))
                                un += 1
                        pss.append(ps)
                    emit_qkv_col(t, 2 * cpi, pss[0][:, 0:256])
                    emit_qkv_col(t, 2 * cpi + 1, pss[1][:, 0:256])
                    while un < (len(uq) * (cpn + 1)) // 4:
                        emit_score_exp(emit_score(*uq[un]))
                        un += 1

                if t >= 1:
                    emit_pv(t - 1)
                if t >= 2:
                    u = t - 2
                    ybig = ybp.tile([128, 4096], bf16, tag="yb", name="yb")
                    for oc in range(NK):
                        emit_oproj_chain(u, oc, ybig)
                        if oc % 4 == 3:
                            sl = slice((oc - 3) * 256, (oc + 1) * 256)
                            nc.sync.dma_start(
                                out=yT_d[:, u * 4096 + sl.start:
                                         u * 4096 + sl.stop],
                                in_=ybig[:, sl])

            # ---- tail: attn(7) interleaved with o_proj(6), then o_proj(7)
            qi = NT - 1
            units = u7[len(u7) // 2:]
            ybig6 = ybp.tile([128, 4096], bf16, tag="yb", name="yb6")
            ui = 0
            for oc in range(6):
                take = ((oc + 1) * len(units)) // 6 - (oc * len(units)) // 6
                for _ in range(take):
                    emit_score_exp(emit_score(*units[ui]))
                    ui += 1
                emit_oproj_chain(NT - 2, oc, ybig6)
                if oc % 4 == 3:
                    sl = slice((oc - 3) * 256, (oc + 1) * 256)
                    nc.sync.dma_start(
                        out=yT_d[:, (NT - 2) * 4096 + sl.start:
                                 (NT - 2) * 4096 + sl.stop],
                        in_=ybig6[:, sl])
            emit_pv(qi)
            # last 10 o_proj(6) chains fill the norm(7) chain latency;
            # copies go to Act so DVE can run the norm multiplies at once
            for oc in range(6, NK):
                emit_oproj_chain(NT - 2, oc, ybig6, act_only=True)
                if oc % 4 == 3:
                    sl = slice((oc - 3) * 256, (oc + 1) * 256)
                    nc.sync.dma_start(
                        out=yT_d[:, (NT - 2) * 4096 + sl.start:
                                 (NT - 2) * 4096 + sl.stop],
                        in_=ybig6[:, sl])
            ybig7 = ybp.tile([128, 4096], bf16, tag="yb", name="yb7")
            for oc in range(NK):
                emit_oproj_chain(NT - 1, oc, ybig7)
                if oc % 2 == 1:
                    sl = slice((oc - 1) * 256, (oc + 1) * 256)
                    nc.sync.dma_start(
                        out=yT_d[:, (NT - 1) * 4096 + sl.start:
                                 (NT - 1) * 4096 + sl.stop],
                        in_=ybig7[:, sl])

    nc.finalize()
    return nc


def _host_inputs(hidden, W_pack, W_o, conv_k, conv_v):
    """Per-core input maps (all bf16, host-blocked layouts)."""
    bf = ml_dtypes.bfloat16
    pos = np.arange(S, dtype=np.float64)
    inv_freq = 1.0 / (THETA ** (np.arange(0, HD, 2, dtype=np.float64) / HD))
    freqs = np.outer(pos, inv_freq)                       # (S, 64)
    cos = np.cos(freqs).T.astype(np.float32)              # (64, S)
    sin = np.sin(freqs).T.astype(np.float32)
    cs = np.concatenate([cos, cos], axis=0).astype(bf)    # (128, S)
    sn = np.concatenate([sin, sin], axis=0).astype(bf)

    kk = np.arange(128)[:, None]
    qq = np.arange(256)[None, :]

    def double(m):
        return np.concatenate([m, m], axis=1).astype(np.float32)

    t0 = double(np.where(kk <= qq, 0.0, NEG))             # delta = 0
    tm128 = double(np.where(kk <= qq - 128, 0.0, NEG))    # delta = -128
    w896 = double(np.where(qq - kk < 128, 0.0, NEG))      # delta = 896
    w1024 = double(np.where(qq < kk, 0.0, NEG))           # delta = 1024
    msk = np.concatenate([w1024, w896, t0, tm128], axis=1).astype(bf)

    ey6 = np.eye(128, dtype=np.float32).astype(bf)
    one = np.ones((128, 8), dtype=np.float32).astype(bf)

    in_maps = []
    for c in range(NCORES):
        b, g = c // TP, c % TP
        # hidden chunk-k blocked: [p, t*4096 + k*256 + tok]
        hblk = np.ascontiguousarray(
            hidden[b].astype(bf).reshape(NT, 256, NK, 128)
            .transpose(3, 0, 2, 1).reshape(128, NT * 4096))
        wq = W_pack[:, g * 512:(g + 1) * 512]
        wk = W_pack[:, NH * HD + 2 * g * 128: NH * HD + (2 * g + 2) * 128]
        wv = W_pack[:, NH * HD + NKV * HD + 2 * g * 128:
                    NH * HD + NKV * HD + (2 * g + 2) * 128]
        wsel = np.concatenate([wq, wk, wv], axis=1).astype(bf)  # (2048, 1024)
        # col-pair-k blocked: [p, cp*4096 + k*256 + cc]
        wblk = np.ascontiguousarray(
            wsel.reshape(NK, 128, 4, 256).transpose(1, 2, 0, 3)
            .reshape(128, 4 * 4096))
        wo = np.ascontiguousarray(
            W_o[g * 512:(g + 1) * 512, :]).astype(bf)
        cwv = np.empty(8, np.float32)
        for i in range(KVH):
            cwv[2 * i] = conv_k[2 * g + i, 0]
            cwv[2 * i + 1] = conv_k[2 * g + i, 1]
            cwv[4 + 2 * i] = conv_v[2 * g + i, 0]
            cwv[4 + 2 * i + 1] = conv_v[2 * g + i, 1]
        cw = np.broadcast_to(cwv, (128, 8)).astype(np.float32).copy()
        in_maps.append({
            "hT": hblk, "wpk": wblk, "wo": wo, "cs": cs, "sn": sn,
            "cw": cw, "msk": msk, "ey6": ey6, "one": one,
        })
    return in_maps


def run_cores(in_maps, trace=False, **kw):
    from concourse.bass_utils import run_bass_kernel_spmd
    if "nc" not in _CACHE:
        _CACHE["nc"] = _build_program()
    return run_bass_kernel_spmd(_CACHE["nc"], in_maps, list(range(NCORES)),
                                trace=trace, **kw)


def kernel(hidden, W_pack, W_o, conv_k, conv_v):
    hidden = np.asarray(hidden, np.float32)
    W_pack = np.asarray(W_pack, np.float32)
    W_o = np.asarray(W_o, np.float32)
    conv_k = np.asarray(conv_k, np.float32)
    conv_v = np.asarray(conv_v, np.float32)
    in_maps = _host_inputs(hidden, W_pack, W_o, conv_k, conv_v)
    res = run_cores(in_maps)
    out = np.zeros((B, S, H), np.float32)
    for c in range(NCORES):
        b = c // TP
        # yT blocked [p, u*4096 + oc*256 + tok] -> partial [H, S]
        arr = np.asarray(res.results[c]["yT"]).astype(np.float32)
        part = arr.reshape(128, NT, NK, 256).transpose(2, 0, 1, 3).reshape(H, S)
        out[b] += part.T
    return out
